# revision 38
# baseline (speedup 1.0000x reference)
"""Trainium2 Bass kernel for a 3-layer GAT (nn_AzureMLGraphAttentionNetwork).

Distribution strategy (8 NeuronCores, SPMD single program + per-core data):
  - Destination nodes are sharded 1250/core. Each core computes the dense
    feature transform for ITS node slice, then all cores AllGather the
    "record" table (attention logits + transformed features per node).
  - Each core processes only the edges whose destination lands in its
    slice: edges are host-sorted by dst, grouped into 128-dst blocks, and
    the per-edge source records are fetched with dma_gather (SWDGE
    descriptor gather, host-known indices as int16 data).
  - Records are packed: [es (H fp16) | h (D fp8)] rounded to 256B, so the
    per-edge gather moves 768B (layers 1-2) / 256B (layer 3) instead of
    the naive fp32 layout. h is upcast fp8->fp16 on the Scalar engine.
  - ed[dst] is NOT gathered: destinations are local, so the per-edge
    ed value is produced on the PE by a one-hot matmul
        edv[e, :] = S_T[dst(e), e]^T @ ed_local[dst, :]
    where S_T is built on the DVE from a clamp+is_equal range trick
    (edges are dst-sorted, so each dst owns a contiguous edge range).
  - Segment softmax is restructured: no segment-max (values are bounded
    so exp is safe), normalization after aggregation:
        out[d] = (sum_e ex_e * h[src_e]) / (sum_e ex_e)
    Both sums come from the same PE matmuls against one-hot scatter
    matrices built on device via iota==dst compare.

The program is identical on all cores; all per-core differences (node
slice, edge indices, scatter structure) enter as input tensors.
"""
import os
import sys

sys.path.insert(0, "/opt/trn_rl_repo")

import numpy as np

import concourse.bass as bass
import concourse.bacc as bacc
import concourse.mybir as mybir
import concourse.tile as tile
from concourse.bass_utils import run_bass_kernel_spmd

F32 = mybir.dt.float32
F16 = mybir.dt.float16
F8 = mybir.dt.float8e4
I16 = mybir.dt.int16

NEG_SLOPE = 0.2
DEN_EPS = 1e-9
EMPTY_SC = 30000.0  # start sentinel for dst with no edges in a chunk


# --------------------------------------------------------------------------
# Configuration
# --------------------------------------------------------------------------
def full_cfg(t_blk=36):
    return dict(
        N=10000,          # total nodes
        CORES=8,
        NLOC=1250,        # nodes per core
        HEADS=8, F=64,    # layers 1-2 heads
        IN=256, HID=512, OUT=32,
        T_BLK=t_blk,      # edge tiles (128 edges) per 128-dst block
        CHUNK=12,         # tiles per dma_gather chunk (must divide T_BLK;
                          # CHUNK*128 must stay fp16-exact, i.e. <= 2048)
    )


def small_cfg():
    # scaled-down config for fast simulator iteration
    return dict(
        N=2048, CORES=8, NLOC=256,
        HEADS=8, F=64, IN=256, HID=512, OUT=32,
        T_BLK=12, CHUNK=4,
    )


def derived(cfg):
    d = dict(cfg)
    d["MT"] = (cfg["NLOC"] + 127) // 128          # m-tiles per core
    d["NPAD"] = d["MT"] * 128
    d["LASTM"] = cfg["NLOC"] - (d["MT"] - 1) * 128  # rows in last m-tile
    d["B"] = d["MT"]                               # dst blocks per core
    d["CPB"] = cfg["T_BLK"] // cfg["CHUNK"]        # chunks per block
    assert cfg["T_BLK"] % cfg["CHUNK"] == 0
    assert cfg["CHUNK"] * 128 <= 2048              # fp16-exact iota coords
    d["NT"] = d["B"] * cfg["T_BLK"]                # edge tiles per core
    d["NCH"] = d["NT"] // cfg["CHUNK"]             # chunks per core
    d["EPC"] = d["NT"] * 128                       # padded edges per core
    d["IDXC"] = d["EPC"] // 16
    d["IPC"] = cfg["CHUNK"] * 128 // 16            # idx cols per chunk
    d["CE"] = cfg["CHUNK"] * 128                   # edges per chunk

    H, HID, OUT = cfg["HEADS"], cfg["HID"], cfg["OUT"]

    def rec_round(x):  # record length (fp16 elems): multiple of 128 (=256B)
        return ((x + 127) // 128) * 128

    # layer descriptors: K=input dim, D=output dim, H=heads
    # FP8: h stored as D fp8 bytes inside the fp16-typed record
    d["L"] = [
        dict(K=cfg["IN"], D=HID, H=H, FP8=True),
        dict(K=HID, D=HID, H=H, FP8=True),
        dict(K=HID, D=OUT, H=1, FP8=False),
    ]
    for L in d["L"]:
        L["KT"] = L["K"] // 128
        L["FH"] = L["D"] // L["H"]                 # features per head
        # record layout (fp16 elems): [es H | h] ; h is D/2 fp16-slots when
        # fp8-packed, D slots when fp16.
        L["HW16"] = L["D"] // 2 if L["FP8"] else L["D"]
        L["REC"] = rec_round(L["H"] + L["HW16"])
    return d


# --------------------------------------------------------------------------
# Host preprocessing
# --------------------------------------------------------------------------
def prep_edges(edge_index, cfg):
    """Per-core edge structure. Returns per-core dicts of:
    src_idx [128, IDXC] i16, dlf [128, NT] f16, sc/ec [128, NCH] f32."""
    d = derived(cfg)
    N, CORES, NLOC = cfg["N"], cfg["CORES"], cfg["NLOC"]
    T_BLK, CHUNK, CPB, CE = cfg["T_BLK"], cfg["CHUNK"], d["CPB"], d["CE"]

    loop = np.arange(N, dtype=np.int64)
    src = np.concatenate([np.asarray(edge_index[0], np.int64), loop])
    dst = np.concatenate([np.asarray(edge_index[1], np.int64), loop])

    out = []
    for c in range(CORES):
        lo, hi = c * NLOC, (c + 1) * NLOC
        m = (dst >= lo) & (dst < hi)
        s_c, d_c = src[m], dst[m] - lo
        order = np.argsort(d_c, kind="stable")
        s_c, d_c = s_c[order], d_c[order]

        e_src = np.zeros(d["EPC"], np.int64)
        dl = np.full(d["EPC"], 999.0, np.float32)
        # remap node id -> row in the split-layout table:
        # region A = [cores x m-tiles 0..MT-2], region B = [cores x last tile]
        SPL = (d["MT"] - 1) * 128
        LAST = NLOC - SPL
        def remap(n):
            cc_, rr = n // NLOC, n % NLOC
            return np.where(rr < SPL, cc_ * SPL + rr,
                            CORES * SPL + cc_ * LAST + (rr - SPL))
        blk_of = d_c // 128
        for b in range(d["B"]):
            sel = blk_of == b
            nb = int(sel.sum())
            cap = T_BLK * 128
            assert nb <= cap, f"block overflow: core {c} blk {b}: {nb} > {cap}"
            base = b * cap
            e_src[base:base + nb] = remap(s_c[sel])
            dl[base:base + nb] = (d_c[sel] - b * 128).astype(np.float32)
            # padding: gather row 0 (finite data), dl=999 -> zero scatter row

        def wrap_idx(a):
            w = np.zeros((16, d["IDXC"]), np.int16)
            w[np.arange(d["EPC"]) % 16, np.arange(d["EPC"]) // 16] = a.astype(np.int16)
            return np.tile(w, (8, 1))

        dlw = np.zeros((128, d["NT"]), np.float32)
        ii = np.arange(d["EPC"])
        dlw[ii % 128, ii // 128] = dl

        out.append(dict(src_idx=wrap_idx(e_src),
                        dlf=dlw.astype(np.float16)))
    return out


def prep_weights(inputs, cfg):
    """Shared (replicated) weight inputs, prepacked for the program."""
    d = derived(cfg)
    H, F = cfg["HEADS"], cfg["F"]

    def wa(W, a_s, a_d, heads, fh):
        Wr = np.asarray(W, np.float32).reshape(W.shape[0], heads, fh)
        WAs = np.einsum("ihf,hf->ih", Wr, np.asarray(a_s, np.float32))
        WAd = np.einsum("ihf,hf->ih", Wr, np.asarray(a_d, np.float32))
        return np.concatenate([WAd, WAs], axis=1)  # order [ed | es]

    out = {}
    specs = [
        ("1", inputs["W1"], inputs["a1s"], inputs["a1d"], inputs["b1"], H, F),
        ("2", inputs["W2"], inputs["a2s"], inputs["a2d"], inputs["b2"], H, F),
        ("3", inputs["W3"], inputs["a3s"], inputs["a3d"], inputs["b3"], 1, cfg["OUT"]),
    ]
    for i, (tag, W, a_s, a_d, b, heads, fh) in enumerate(specs):
        L = d["L"][i]
        W = np.asarray(W, np.float32)
        out[f"W{tag}p"] = W.reshape(L["KT"], 128, L["D"]).astype(np.float16)
        out[f"WA{tag}p"] = (wa(W, a_s, a_d, heads, fh)
                            .reshape(L["KT"], 128, 2 * L["H"]).astype(np.float16))
        out[f"brep{tag}"] = np.broadcast_to(
            np.asarray(b, np.float32), (128, L["D"])).copy()
    out["ident"] = np.eye(128, dtype=np.float32)
    out["ident16"] = np.eye(128, dtype=np.float16)
    out["iota_c"] = np.broadcast_to(
        np.arange(128, dtype=np.float16), (128, cfg["CHUNK"], 128)).copy()
    return out


def prep_x(x, cfg, core):
    """Per-core transposed input slice: [KT1, 128, NPAD] f16."""
    d = derived(cfg)
    NLOC, NPAD = cfg["NLOC"], d["NPAD"]
    xs = np.zeros((NPAD, cfg["IN"]), np.float32)
    xs[:NLOC] = np.asarray(x[core * NLOC:(core + 1) * NLOC], np.float32)
    return np.ascontiguousarray(
        xs.T.reshape(d["L"][0]["KT"], 128, NPAD)).astype(np.float16)


# --------------------------------------------------------------------------
# Program builder
# --------------------------------------------------------------------------
def build_program(cfg, has_bias=(False, False, False)):
    d = derived(cfg)
    N, CORES = cfg["N"], cfg["CORES"]
    NLOC, MT, NPAD, LASTM = cfg["NLOC"], d["MT"], d["NPAD"], d["LASTM"]
    B, T_BLK, CHUNK, CPB = d["B"], cfg["T_BLK"], cfg["CHUNK"], d["CPB"]
    NCH, IPC, CE = d["NCH"], d["IPC"], d["CE"]
    Ls = d["L"]

    nc = bacc.Bacc(num_devices=CORES, num_swdge_queues=4)

    # ---- external inputs
    xT0 = nc.dram_tensor("xT0", [Ls[0]["KT"], 128, NPAD], F16, kind="ExternalInput")
    Wp, WAp, brep = [], [], []
    for i, L in enumerate(Ls):
        t = str(i + 1)
        Wp.append(nc.dram_tensor(f"W{t}p", [L["KT"], 128, L["D"]], F16, kind="ExternalInput"))
        WAp.append(nc.dram_tensor(f"WA{t}p", [L["KT"], 128, 2 * L["H"]], F16, kind="ExternalInput"))
        brep.append(nc.dram_tensor(f"brep{t}", [128, L["D"]], F32, kind="ExternalInput"))
    src_idx = nc.dram_tensor("src_idx", [128, d["IDXC"]], I16, kind="ExternalInput")
    dlf = nc.dram_tensor("dlf", [128, d["NT"]], F16, kind="ExternalInput")
    iota_c = nc.dram_tensor("iota_c", [128, CHUNK, 128], F16, kind="ExternalInput")
    ident = nc.dram_tensor("ident", [128, 128], F32, kind="ExternalInput")
    ident16 = nc.dram_tensor("ident16", [128, 128], F16, kind="ExternalInput")
    y_out = nc.dram_tensor("y", [NLOC, cfg["OUT"]], F32, kind="ExternalOutput")

    # ---- internal DRAM record tables
    warm_in = nc.dram_tensor("warm_in", [1, 128], F16)
    warm_out = nc.dram_tensor("warm_out", [CORES, 128], F16, addr_space="Shared")
    rec_slice = [nc.dram_tensor(f"rec_slice{i}", [NLOC, L["REC"]], F16)
                 for i, L in enumerate(Ls)]
    rec_table = [nc.dram_tensor(f"rec_table{i}", [N, L["REC"]], F16,
                                addr_space="Shared")
                 for i, L in enumerate(Ls)]

    groups = [list(range(CORES))]

    with tile.TileContext(nc) as tc:
        with (
            tc.tile_pool(name="const", bufs=1) as const,
            tc.tile_pool(name="xt", bufs=1) as xtp,
            tc.tile_pool(name="work", bufs=1) as work,
            tc.tile_pool(name="gp", bufs=3) as gp,
            tc.tile_pool(name="gw", bufs=2) as gwp,
            tc.tile_pool(name="st", bufs=2) as stp,
            tc.tile_pool(name="small", bufs=3) as small,
            tc.tile_pool(name="ps", bufs=2, space="PSUM") as ps,
        ):
            nidx_reg = nc.gpsimd.to_reg(CE)
            nidx_reg2 = nc.gpsimd.to_reg(2 * CE)

            # absorb the one-time collective trigger warmup (~38us) before
            # the first real AllGather
            wz = const.tile([1, 128], F16, name="wz", tag="wz")
            nc.vector.memset(wz[:], 0.0)
            nc.sync.dma_start(warm_in[:], wz[:])
            nc.gpsimd.collective_compute(
                "AllGather", mybir.AluOpType.bypass,
                replica_groups=[list(range(CORES))],
                ins=[warm_in[:]], outs=[warm_out[:]])

            # ---- constants into SBUF
            def load_const(ap, shape, dt=F32, name="cst"):
                t = const.tile(shape, dt, name=name, tag=name)
                nc.sync.dma_start(t[:], ap[:])
                return t

            src_t = load_const(src_idx, [128, d["IDXC"]], I16, name="src_t")
            dlf_t = load_const(dlf, [128, d["NT"]], F16, name="dlf_t")
            iota_t = load_const(iota_c, [128, CHUNK, 128], F16, name="iota_t")
            id_t = load_const(ident, [128, 128], name="id_t")
            id16_t = load_const(ident16, [128, 128], F16, name="id16_t")

            def load_kt(ap, kt, width, name):  # [kt,128,w] dram -> [128,kt,w]
                t = const.tile([128, kt, width], F16, name=name, tag=name)
                nc.sync.dma_start(t[:], ap.rearrange("k p w -> p k w"))
                return t

            W_t = [load_kt(Wp[i], Ls[i]["KT"], Ls[i]["D"], f"W_t{i}")
                   for i in range(3)]
            WA_t = [load_kt(WAp[i], Ls[i]["KT"], 2 * Ls[i]["H"], f"WA_t{i}")
                    for i in range(3)]
            b_t = [load_const(brep[i], [128, Ls[i]["D"]], name=f"b_t{i}")
                   if has_bias[i] else None for i in range(3)]

            # layer-1 x^T
            xT = [xtp.tile([128, NPAD], F16, tag=f"xt{k}", name=f"xTa{k}")
                  for k in range(Ls[0]["KT"])]
            for k in range(Ls[0]["KT"]):
                nc.sync.dma_start(xT[k][:], xT0[k][:])

            y_sb = work.tile([128, MT, cfg["OUT"]], F32, tag="y_sb")
            edl = work.tile([128, MT, 8], F16, tag="edl")
            gidx = [0]  # global gather counter: queue must track the DMASW
                        # sem lane rotation (emission order mod 8 -> mod 4)

            rec_sbs = {}

            def phase_a(li, m, xTl):
                """Dense transform + record/edl write for one m-tile."""
                L = Ls[li]
                H, D, KT, REC, HW16 = L["H"], L["D"], L["KT"], L["REC"], L["HW16"]
                rec_sb = rec_sbs[li]
                ph = ps.tile([128, D], F32, tag="big")
                pe = ps.tile([128, 2 * H], F32, tag="sm")
                lhs = [xTl[k][:, m * 128:(m + 1) * 128] for k in range(KT)]
                for k in range(KT):
                    nc.tensor.matmul(ph[:], lhs[k], W_t[li][:, k, :],
                                     start=(k == 0), stop=(k == KT - 1))
                for k in range(KT):
                    nc.tensor.matmul(pe[:], lhs[k], WA_t[li][:, k, :],
                                     start=(k == 0), stop=(k == KT - 1))
                if L["FP8"]:
                    nc.scalar.activation(
                        rec_sb[:, m, H:H + HW16].bitcast(F8), ph[:],
                        mybir.ActivationFunctionType.Identity)
                else:
                    nc.scalar.activation(
                        rec_sb[:, m, H:H + HW16], ph[:],
                        mybir.ActivationFunctionType.Identity)
                nc.scalar.activation(rec_sb[:, m, 0:H], pe[:, H:2 * H],
                                     mybir.ActivationFunctionType.Identity)
                nc.scalar.activation(edl[:, m, 0:H], pe[:, 0:H],
                                     mybir.ActivationFunctionType.Identity)
                # record write for this m-tile: rows n = m*128 + p
                rs = rec_slice[li]
                if m < MT - 1:
                    nc.sync.dma_start(
                        rs[m * 128:(m + 1) * 128, :]
                        .rearrange("(u p) c -> p u c", p=128),
                        rec_sb[:, m:m + 1, :])
                else:
                    nc.sync.dma_start(rs[m * 128:NLOC, :],
                                      rec_sb[0:LASTM, m, :])

            def alloc_rec(li):
                L = Ls[li]
                rec_sb = work.tile([128, MT, L["REC"]], F16, tag=f"rec_sb{li}",
                                   name=f"rec_sb{li}")
                rec_sbs[li] = rec_sb
                PAD0 = L["H"] + L["HW16"]
                if L["REC"] > PAD0:
                    nc.vector.memset(rec_sb[:, :, PAD0:L["REC"]], 0.0)

            # layer-1 dense phase up front
            alloc_rec(0)
            for m in range(MT):
                phase_a(0, m, xT)

            for li, L in enumerate(Ls):
                H, D, FH, KT, REC = L["H"], L["D"], L["FH"], L["KT"], L["REC"]
                HW16 = L["HW16"]

                # ---------------- Phase B: AllGather -----------------------
                # split: m-tiles 0..MT-2 are written early (their phase_a ran
                # inside the previous layer's loop), so that collective can
                # overlap the previous layer's tail; only the last m-tile's
                # small collective is on the critical path.
                SPL = (MT - 1) * 128
                nc.gpsimd.collective_compute(
                    "AllGather", mybir.AluOpType.bypass,
                    replica_groups=groups,
                    ins=[rec_slice[li][0:SPL, :]],
                    outs=[rec_table[li][0:CORES * SPL, :]],
                )
                nc.gpsimd.collective_compute(
                    "AllGather", mybir.AluOpType.bypass,
                    replica_groups=groups,
                    ins=[rec_slice[li][SPL:NLOC, :]],
                    outs=[rec_table[li][CORES * SPL:N, :]],
                )

                # ---------------- Phase C: edge pipeline -------------------
                if li < 2:
                    KTn = Ls[li + 1]["KT"]
                    xTn = [xtp.tile([128, NPAD], F16, tag=f"xt{k}",
                                    name=f"xTn{li}_{k}") for k in range(KTn)]
                    alloc_rec(li + 1)
                # grouped gathers: 4 chunks share one tile so all 4 become
                # ready together and the Q7 DGE generates them in one launch
                GRP = 4
                # narrow layer-3 records fit 2 chunks per gather slot -> half
                # the Q7 launch overhead there
                CPG = 2 if (li == 2 and NCH % 2 == 0
                            and 2 * CHUNK * REC <= CHUNK * 384) else 1
                gtiles = {}
                for c0 in range(0, NCH, CPG * GRP):
                    G4 = gp.tile([128, GRP, CHUNK * 384], F16, tag="G4")
                    for j in range(GRP):
                        cs = c0 + j * CPG
                        if cs >= NCH:
                            break
                        q = gidx[0] % 4
                        gidx[0] += 1
                        gv = (G4[:, j, 0:CPG * CHUNK * REC]
                              .rearrange("p (t r) -> p t r", t=CPG * CHUNK))
                        for k in range(CPG):
                            gtiles[cs + k] = gv[:, k * CHUNK:(k + 1) * CHUNK, :]
                        nc.gpsimd.dma_gather(
                            out_ap=gv,
                            in_ap=rec_table[li][:, :],
                            idxs_ap=src_t[:, cs * IPC:(cs + CPG) * IPC],
                            num_idxs=CPG * CE,
                            num_idxs_reg=nidx_reg if CPG == 1 else nidx_reg2,
                            elem_size=REC,
                            single_packet=False,
                            queue_num=q,
                        )
                # stage1(c): S_T build (DVE), S = S_T^T (PE+Scalar), edv (PE)
                staged = {}

                def stage1(c):
                    # S[e, dstoff] one-hot directly on DVE
                    S = stp.tile([128, CHUNK, 128], F16, tag="S")
                    nc.vector.tensor_tensor(
                        S[:], iota_t[:],
                        dlf_t[:, c * CHUNK:(c + 1) * CHUNK, None]
                        .broadcast_to((128, CHUNK, 128)),
                        mybir.AluOpType.is_equal)
                    # S_T = S^T via PE transposes + Scalar PSUM->SBUF copies
                    ST = stp.tile([128, CE], F16, tag="ST")
                    for j in range(CHUNK // 4):
                        tp4 = ps.tile([128, 4, 128], F16, tag="tp")
                        for t4 in range(4):
                            t = j * 4 + t4
                            nc.tensor.transpose(
                                tp4[:, t4, :], S[:, t, :], id16_t[:])
                        nc.scalar.activation(
                            ST[:, j * 512:(j + 1) * 512]
                            .rearrange("p (a b) -> p a b", a=4),
                            tp4[:],
                            mybir.ActivationFunctionType.Identity)
                    edv = ps.tile([128, CHUNK, H], F32, tag="edv")
                    for t in range(CHUNK):
                        nc.tensor.matmul(
                            edv[:, t, :],
                            ST[:, t * 128:(t + 1) * 128],
                            edl[:, c // CPB, 0:H],
                            start=True, stop=True)
                    staged[c] = (S, edv)

                def emit_epi(blk, acc, den):
                    dene = small.tile([128, H], F32, tag="dene")
                    nc.vector.tensor_scalar_add(dene[:], den[:], DEN_EPS)
                    recip = small.tile([128, H], F32, tag="recip")
                    nc.vector.reciprocal(recip[:], dene[:])
                    if li < 2:
                        h_blk = small.tile([128, D], F32, tag="h_blk",
                                           name=f"h_blk{li}")
                        oview = h_blk[:].rearrange("p (h f) -> p h f", h=H)
                        nc.vector.tensor_tensor(
                            oview, acc[:].rearrange("p (h f) -> p h f", h=H),
                            recip[:, :, None].broadcast_to((128, H, FH)),
                            mybir.AluOpType.mult)
                        if has_bias[li]:
                            nc.vector.tensor_tensor(
                                h_blk[:], h_blk[:], b_t[li][:],
                                mybir.AluOpType.add)
                        nc.scalar.activation(h_blk[:], h_blk[:],
                                             mybir.ActivationFunctionType.Relu)
                        # next layer x^T for this m-tile + its dense phase
                        for k in range(KTn):
                            tp = ps.tile([128, 128], F32, tag="tp")
                            nc.tensor.transpose(
                                tp[:], h_blk[:, k * 128:(k + 1) * 128], id_t[:])
                            nc.scalar.activation(
                                xTn[k][:, blk * 128:(blk + 1) * 128], tp[:],
                                mybir.ActivationFunctionType.Identity)
                        phase_a(li + 1, blk, xTn)
                    else:
                        if has_bias[li]:
                            t3 = small.tile([128, cfg["OUT"]], F32, tag="t3")
                            nc.scalar.activation(
                                t3[:], acc[:],
                                mybir.ActivationFunctionType.Identity,
                                scale=recip[:, 0:1])
                            nc.vector.tensor_tensor(y_sb[:, blk, :], t3[:],
                                                    b_t[li][:],
                                                    mybir.AluOpType.add)
                        else:
                            nc.scalar.activation(
                                y_sb[:, blk, :], acc[:],
                                mybir.ActivationFunctionType.Identity,
                                scale=recip[:, 0:1])

                stage1(0)
                pending = None
                for blk in range(B):
                    acc = ps.tile([128, D], F32, tag="big")
                    den = ps.tile([128, H], F32, tag="sm")
                    for cc in range(CPB):
                        c = blk * CPB + cc
                        G = gtiles[c]
                        S, edv = staged.pop(c)
                        # ex = exp(leaky_relu(es[src] + ed[dst]))
                        z = small.tile([128, CHUNK, H], F32, tag="z")
                        nc.vector.tensor_tensor(
                            z[:], G[:, :, 0:H], edv[:],
                            mybir.AluOpType.add)
                        z2 = small.tile([128, CHUNK, H], F32, tag="z2")
                        nc.vector.scalar_tensor_tensor(
                            z2[:], z[:], NEG_SLOPE, z[:],
                            mybir.AluOpType.mult, mybir.AluOpType.max)
                        ex = small.tile([128, CHUNK, H], F16, tag="ex")
                        nc.scalar.activation(
                            ex[:].rearrange("p a b -> p (a b)"),
                            z2[:].rearrange("p a b -> p (a b)"),
                            mybir.ActivationFunctionType.Exp)
                        # build next chunk's one-hots while Scalar runs exp
                        if c + 1 < NCH:
                            stage1(c + 1)
                        # fused h upcast (fp8) + ex weighting in one DVE op
                        Gh = gwp.tile([128, CHUNK, D], F16, tag="Gh")
                        hsrc = (G[:, :, H:H + HW16].bitcast(F8) if L["FP8"]
                                else G[:, :, H:H + HW16])
                        nc.vector.tensor_tensor(
                            Gh[:].rearrange("p t (h f) -> p t h f", h=H),
                            hsrc.rearrange("p t (h f) -> p t h f", h=H),
                            ex[:, :, :, None].broadcast_to((128, CHUNK, H, FH)),
                            mybir.AluOpType.mult)
                        if cc == 0 and pending is not None:
                            emit_epi(*pending)
                            pending = None
                        for t in range(CHUNK):
                            first = (cc == 0 and t == 0)
                            last = (cc == CPB - 1 and t == CHUNK - 1)
                            nc.tensor.matmul(acc[:], S[:, t, :],
                                             Gh[:, t, :],
                                             start=first, stop=last)
                            nc.tensor.matmul(den[:], S[:, t, :], ex[:, t, :],
                                             start=first, stop=last)

                    # epilogue deferred one block (emitted inside next
                    # block's loop so DVE never stalls on this block's PSUM)
                    pending = (blk, acc, den)
                if pending is not None:
                    emit_epi(*pending)
                    pending = None

                # final log-softmax over all m-tiles in one batch (layer 3)
                if li == 2:
                    mxB = small.tile([128, MT, 1], F32, tag="mxB")
                    nc.vector.tensor_reduce(mxB[:], y_sb[:], mybir.AxisListType.X,
                                            mybir.AluOpType.max)
                    nc.vector.tensor_tensor(
                        y_sb[:], y_sb[:],
                        mxB[:, :, 0:1].broadcast_to((128, MT, cfg["OUT"])),
                        mybir.AluOpType.subtract)
                    escB = small.tile([128, MT, cfg["OUT"]], F32, tag="escB")
                    nc.scalar.activation(
                        escB[:].rearrange("p a b -> p (a b)"),
                        y_sb[:].rearrange("p a b -> p (a b)"),
                        mybir.ActivationFunctionType.Exp)
                    smB = small.tile([128, MT, 1], F32, tag="smB")
                    nc.vector.tensor_reduce(smB[:], escB[:], mybir.AxisListType.X,
                                            mybir.AluOpType.add)
                    lnB = small.tile([128, MT, 1], F32, tag="lnB")
                    nc.scalar.activation(lnB[:, :, 0], smB[:, :, 0],
                                         mybir.ActivationFunctionType.Ln)
                    nc.vector.tensor_tensor(
                        y_sb[:], y_sb[:],
                        lnB[:, :, 0:1].broadcast_to((128, MT, cfg["OUT"])),
                        mybir.AluOpType.subtract)

            # ---- output
            full = MT - 1
            if full:
                nc.sync.dma_start(
                    y_out[0:full * 128, :].rearrange("(m p) c -> p m c", p=128),
                    y_sb[:, 0:full, :])
            nc.sync.dma_start(y_out[full * 128:NLOC, :], y_sb[0:LASTM, full, :])

    nc.compile()
    return nc


# --------------------------------------------------------------------------
# Host-side emulation of the exact device algorithm (for testing)
# --------------------------------------------------------------------------
def emulate(inputs, cfg):
    import ml_dtypes
    d = derived(cfg)
    x = np.asarray(inputs["x"], np.float32)
    ei = np.asarray(inputs["edge_index"])
    N = cfg["N"]
    loop = np.arange(N, dtype=np.int64)
    src = np.concatenate([np.asarray(ei[0], np.int64), loop])
    dst = np.concatenate([np.asarray(ei[1], np.int64), loop])

    W = [np.asarray(inputs[f"W{i}"], np.float32) for i in (1, 2, 3)]
    As = [np.asarray(inputs[f"a{i}s"], np.float32) for i in (1, 2, 3)]
    Ad = [np.asarray(inputs[f"a{i}d"], np.float32) for i in (1, 2, 3)]
    bs = [np.asarray(inputs[f"b{i}"], np.float32) for i in (1, 2, 3)]

    def q16(a): return np.asarray(a, np.float16).astype(np.float32)
    def q8(a): return np.asarray(a, ml_dtypes.float8_e4m3).astype(np.float32)

    h = x
    for li, L in enumerate(d["L"]):
        Wr = W[li].reshape(L["K"], L["H"], L["FH"])
        WAs = np.einsum("ihf,hf->ih", Wr, As[li])
        WAd = np.einsum("ihf,hf->ih", Wr, Ad[li])
        h16 = q16(h)
        hh = h16 @ q16(W[li])
        es = q16(h16 @ q16(WAs))
        ed = q16(h16 @ q16(WAd))
        hq = q8(hh) if L["FP8"] else q16(hh)
        z = es[src] + ed[dst]
        ex = q16(np.exp(np.maximum(z, NEG_SLOPE * z)))
        gq = q16(hq[src].reshape(-1, L["H"], L["FH"]) * ex[:, :, None])
        acc = np.zeros((N, L["H"], L["FH"]), np.float64)
        den = np.zeros((N, L["H"]), np.float64)
        np.add.at(acc, dst, gq)
        np.add.at(den, dst, ex)
        out = (acc / (den[:, :, None] + DEN_EPS)).reshape(N, L["D"]).astype(np.float32)
        out = out + bs[li]
        if li < 2:
            h = np.maximum(out, 0.0)
        else:
            h = out
    m = h.max(axis=1, keepdims=True)
    s = h - m
    return s - np.log(np.exp(s).sum(axis=1, keepdims=True))


# --------------------------------------------------------------------------
# In-map assembly + entry point
# --------------------------------------------------------------------------
def build_in_maps(inputs, cfg):
    shared = prep_weights(inputs, cfg)
    percore = prep_edges(inputs["edge_index"], cfg)
    in_maps = []
    for c in range(cfg["CORES"]):
        m = dict(shared)
        m.update(percore[c])
        m["xT0"] = prep_x(np.asarray(inputs["x"], np.float32), cfg, c)
        in_maps.append(m)
    return in_maps


def pick_t_blk(edge_index, cfg):
    """Smallest CHUNK-multiple tile count that fits the largest dst block."""
    N, CORES, NLOC = cfg["N"], cfg["CORES"], cfg["NLOC"]
    dst = np.concatenate([np.asarray(edge_index[1], np.int64),
                          np.arange(N, dtype=np.int64)])
    blk = (dst % NLOC) // 128 + (dst // NLOC) * ((NLOC + 127) // 128)
    mx = int(np.bincount(blk).max())
    ch = cfg["CHUNK"]
    tiles = (mx + 127) // 128
    return max(((tiles + ch - 1) // ch) * ch, ch)


_PROGRAM_CACHE = {}
LAST_EXEC_NS = None


def kernel(**inputs):
    global LAST_EXEC_NS
    cfg = full_cfg()
    t_blk = pick_t_blk(inputs["edge_index"], cfg)
    cfg = full_cfg(t_blk)
    has_bias = tuple(bool(np.any(np.asarray(inputs[f"b{i}"]))) for i in (1, 2, 3))
    key = ("full", t_blk, has_bias)
    if key not in _PROGRAM_CACHE:
        _PROGRAM_CACHE[key] = build_program(cfg, has_bias)
    nc = _PROGRAM_CACHE[key]
    in_maps = build_in_maps(inputs, cfg)
    res = run_bass_kernel_spmd(nc, in_maps, core_ids=list(range(cfg["CORES"])))
    LAST_EXEC_NS = res.exec_time_ns
    y = np.concatenate([res.results[c]["y"] for c in range(cfg["CORES"])], axis=0)
    return y.astype(np.float32)


def time_kernel(inputs, iters=0):
    """On-device exec time from the NTFF hardware profile (core 0)."""
    import shutil, tempfile
    cfg = full_cfg(pick_t_blk(inputs["edge_index"], full_cfg()))
    has_bias = tuple(bool(np.any(np.asarray(inputs[f"b{i}"]))) for i in (1, 2, 3))
    key = ("full", cfg["T_BLK"], has_bias)
    if key not in _PROGRAM_CACHE:
        _PROGRAM_CACHE[key] = build_program(cfg, has_bias)
    nc = _PROGRAM_CACHE[key]
    in_maps = build_in_maps(inputs, cfg)
    tmpdir = tempfile.mkdtemp(prefix="gat_trace_")
    res = run_bass_kernel_spmd(nc, in_maps, core_ids=list(range(cfg["CORES"])),
                               trace=True, tmpdir=tmpdir, trace_cores=[0])
    return dict(est_exec_s=(res.exec_time_ns or 0) * 1e-9,
                trace_dir=tmpdir,
                profile_json=res.profile_json)


if __name__ == "__main__":
    # quick smoke: build the full program
    nc = build_program(full_cfg())
    print("program built ok")


# revision 39
# speedup vs baseline: 1.1126x; 1.1126x over previous
"""Trainium2 Bass kernel for a 3-layer GAT (nn_AzureMLGraphAttentionNetwork).

Distribution strategy (8 NeuronCores, SPMD single program + per-core data):
  - Destination nodes are sharded 1250/core. Each core computes the dense
    feature transform for ITS node slice, then all cores AllGather the
    "record" table (attention logits + transformed features per node).
  - Each core processes only the edges whose destination lands in its
    slice: edges are host-sorted by dst, grouped into 128-dst blocks, and
    the per-edge source records are fetched with dma_gather (SWDGE
    descriptor gather, host-known indices as int16 data).
  - Records are packed: [es (H fp16) | h (D fp8)] rounded to 256B, so the
    per-edge gather moves 768B (layers 1-2) / 256B (layer 3) instead of
    the naive fp32 layout. h is upcast fp8->fp16 on the Scalar engine.
  - ed[dst] is NOT gathered: destinations are local, so the per-edge
    ed value is produced on the PE by a one-hot matmul
        edv[e, :] = S_T[dst(e), e]^T @ ed_local[dst, :]
    where S_T is built on the DVE from a clamp+is_equal range trick
    (edges are dst-sorted, so each dst owns a contiguous edge range).
  - Segment softmax is restructured: no segment-max (values are bounded
    so exp is safe), normalization after aggregation:
        out[d] = (sum_e ex_e * h[src_e]) / (sum_e ex_e)
    Both sums come from the same PE matmuls against one-hot scatter
    matrices built on device via iota==dst compare.

The program is identical on all cores; all per-core differences (node
slice, edge indices, scatter structure) enter as input tensors.
"""
import os
import sys

sys.path.insert(0, "/opt/trn_rl_repo")

import numpy as np

import concourse.bass as bass
import concourse.bacc as bacc
import concourse.mybir as mybir
import concourse.tile as tile
from concourse.bass_utils import run_bass_kernel_spmd

F32 = mybir.dt.float32
F16 = mybir.dt.float16
F8 = mybir.dt.float8e4
I16 = mybir.dt.int16

NEG_SLOPE = 0.2
DEN_EPS = 1e-9
EMPTY_SC = 30000.0  # start sentinel for dst with no edges in a chunk


# --------------------------------------------------------------------------
# Configuration
# --------------------------------------------------------------------------
def full_cfg(t_blk=36):
    return dict(
        N=10000,          # total nodes
        CORES=8,
        NLOC=1250,        # nodes per core
        HEADS=8, F=64,    # layers 1-2 heads
        IN=256, HID=512, OUT=32,
        T_BLK=t_blk,      # edge tiles (128 edges) per 128-dst block
        CHUNK=12,         # tiles per dma_gather chunk (must divide T_BLK;
                          # CHUNK*128 must stay fp16-exact, i.e. <= 2048)
    )


def small_cfg():
    # scaled-down config for fast simulator iteration
    return dict(
        N=2048, CORES=8, NLOC=256,
        HEADS=8, F=64, IN=256, HID=512, OUT=32,
        T_BLK=12, CHUNK=4,
    )


def derived(cfg):
    d = dict(cfg)
    d["MT"] = (cfg["NLOC"] + 127) // 128          # m-tiles per core
    d["NPAD"] = d["MT"] * 128
    d["LASTM"] = cfg["NLOC"] - (d["MT"] - 1) * 128  # rows in last m-tile
    d["B"] = d["MT"]                               # dst blocks per core
    d["CPB"] = cfg["T_BLK"] // cfg["CHUNK"]        # chunks per block
    assert cfg["T_BLK"] % cfg["CHUNK"] == 0
    assert cfg["CHUNK"] * 128 <= 2048              # fp16-exact iota coords
    d["NT"] = d["B"] * cfg["T_BLK"]                # edge tiles per core
    d["NCH"] = d["NT"] // cfg["CHUNK"]             # chunks per core
    d["EPC"] = d["NT"] * 128                       # padded edges per core
    d["IDXC"] = d["EPC"] // 16
    d["IPC"] = cfg["CHUNK"] * 128 // 16            # idx cols per chunk
    d["CE"] = cfg["CHUNK"] * 128                   # edges per chunk

    H, HID, OUT = cfg["HEADS"], cfg["HID"], cfg["OUT"]

    def rec_round(x):  # record length (fp16 elems): multiple of 128 (=256B)
        return ((x + 127) // 128) * 128

    # layer descriptors: K=input dim, D=output dim, H=heads
    # FP8: h stored as D fp8 bytes inside the fp16-typed record
    d["L"] = [
        dict(K=cfg["IN"], D=HID, H=H, FP8=True),
        dict(K=HID, D=HID, H=H, FP8=True),
        dict(K=HID, D=OUT, H=1, FP8=False),
    ]
    for L in d["L"]:
        L["KT"] = L["K"] // 128
        L["FH"] = L["D"] // L["H"]                 # features per head
        # record layout (fp16 elems): [es H | h] ; h is D/2 fp16-slots when
        # fp8-packed, D slots when fp16.
        L["HW16"] = L["D"] // 2 if L["FP8"] else L["D"]
        L["REC"] = rec_round(L["H"] + L["HW16"])
    return d


# --------------------------------------------------------------------------
# Host preprocessing
# --------------------------------------------------------------------------
def prep_edges(edge_index, cfg):
    """Per-core edge structure. Returns per-core dicts of:
    src_idx [128, IDXC] i16, dlf [128, NT] f16, sc/ec [128, NCH] f32."""
    d = derived(cfg)
    N, CORES, NLOC = cfg["N"], cfg["CORES"], cfg["NLOC"]
    T_BLK, CHUNK, CPB, CE = cfg["T_BLK"], cfg["CHUNK"], d["CPB"], d["CE"]

    loop = np.arange(N, dtype=np.int64)
    src = np.concatenate([np.asarray(edge_index[0], np.int64), loop])
    dst = np.concatenate([np.asarray(edge_index[1], np.int64), loop])

    out = []
    for c in range(CORES):
        lo, hi = c * NLOC, (c + 1) * NLOC
        m = (dst >= lo) & (dst < hi)
        s_c, d_c = src[m], dst[m] - lo
        order = np.argsort(d_c, kind="stable")
        s_c, d_c = s_c[order], d_c[order]

        e_src = np.zeros(d["EPC"], np.int64)
        dl = np.full(d["EPC"], 999.0, np.float32)
        # remap node id -> row in the split-layout table:
        # region A = [cores x m-tiles 0..MT-2], region B = [cores x last tile]
        SPL = (d["MT"] - 1) * 128
        LAST = NLOC - SPL
        def remap(n):
            cc_, rr = n // NLOC, n % NLOC
            return np.where(rr < SPL, cc_ * SPL + rr,
                            CORES * SPL + cc_ * LAST + (rr - SPL))
        blk_of = d_c // 128
        for b in range(d["B"]):
            sel = blk_of == b
            nb = int(sel.sum())
            cap = T_BLK * 128
            assert nb <= cap, f"block overflow: core {c} blk {b}: {nb} > {cap}"
            base = b * cap
            e_src[base:base + nb] = remap(s_c[sel])
            dl[base:base + nb] = (d_c[sel] - b * 128).astype(np.float32)
            # padding: gather row 0 (finite data), dl=999 -> zero scatter row

        def wrap_idx(a):
            w = np.zeros((16, d["IDXC"]), np.int16)
            w[np.arange(d["EPC"]) % 16, np.arange(d["EPC"]) // 16] = a.astype(np.int16)
            return np.tile(w, (8, 1))

        dlw = np.zeros((128, d["NT"]), np.float32)
        ii = np.arange(d["EPC"])
        dlw[ii % 128, ii // 128] = dl

        out.append(dict(src_idx=wrap_idx(e_src),
                        dlf=dlw.astype(np.float16)))
    return out


def prep_weights(inputs, cfg):
    """Shared (replicated) weight inputs, prepacked for the program."""
    d = derived(cfg)
    H, F = cfg["HEADS"], cfg["F"]

    def wa(W, a_s, a_d, heads, fh):
        Wr = np.asarray(W, np.float32).reshape(W.shape[0], heads, fh)
        WAs = np.einsum("ihf,hf->ih", Wr, np.asarray(a_s, np.float32))
        WAd = np.einsum("ihf,hf->ih", Wr, np.asarray(a_d, np.float32))
        return np.concatenate([WAd, WAs], axis=1)  # order [ed | es]

    out = {}
    specs = [
        ("1", inputs["W1"], inputs["a1s"], inputs["a1d"], inputs["b1"], H, F),
        ("2", inputs["W2"], inputs["a2s"], inputs["a2d"], inputs["b2"], H, F),
        ("3", inputs["W3"], inputs["a3s"], inputs["a3d"], inputs["b3"], 1, cfg["OUT"]),
    ]
    for i, (tag, W, a_s, a_d, b, heads, fh) in enumerate(specs):
        L = d["L"][i]
        W = np.asarray(W, np.float32)
        out[f"W{tag}p"] = W.reshape(L["KT"], 128, L["D"]).astype(np.float16)
        out[f"WA{tag}p"] = (wa(W, a_s, a_d, heads, fh)
                            .reshape(L["KT"], 128, 2 * L["H"]).astype(np.float16))
        out[f"brep{tag}"] = np.broadcast_to(
            np.asarray(b, np.float32), (128, L["D"])).copy()
    out["ident"] = np.eye(128, dtype=np.float32)
    out["ident16"] = np.eye(128, dtype=np.float16)
    out["iota_c"] = np.broadcast_to(
        np.arange(128, dtype=np.float16), (128, cfg["CHUNK"], 128)).copy()
    return out


def prep_x(x, cfg, core):
    """Per-core transposed input slice: [KT1, 128, NPAD] f16."""
    d = derived(cfg)
    NLOC, NPAD = cfg["NLOC"], d["NPAD"]
    xs = np.zeros((NPAD, cfg["IN"]), np.float32)
    xs[:NLOC] = np.asarray(x[core * NLOC:(core + 1) * NLOC], np.float32)
    return np.ascontiguousarray(
        xs.T.reshape(d["L"][0]["KT"], 128, NPAD)).astype(np.float16)


# --------------------------------------------------------------------------
# Program builder
# --------------------------------------------------------------------------
def build_program(cfg, has_bias=(False, False, False)):
    d = derived(cfg)
    N, CORES = cfg["N"], cfg["CORES"]
    NLOC, MT, NPAD, LASTM = cfg["NLOC"], d["MT"], d["NPAD"], d["LASTM"]
    B, T_BLK, CHUNK, CPB = d["B"], cfg["T_BLK"], cfg["CHUNK"], d["CPB"]
    NCH, IPC, CE = d["NCH"], d["IPC"], d["CE"]
    Ls = d["L"]

    nc = bacc.Bacc(num_devices=CORES, num_swdge_queues=4)

    # ---- external inputs
    xT0 = nc.dram_tensor("xT0", [Ls[0]["KT"], 128, NPAD], F16, kind="ExternalInput")
    Wp, WAp, brep = [], [], []
    for i, L in enumerate(Ls):
        t = str(i + 1)
        Wp.append(nc.dram_tensor(f"W{t}p", [L["KT"], 128, L["D"]], F16, kind="ExternalInput"))
        WAp.append(nc.dram_tensor(f"WA{t}p", [L["KT"], 128, 2 * L["H"]], F16, kind="ExternalInput"))
        brep.append(nc.dram_tensor(f"brep{t}", [128, L["D"]], F32, kind="ExternalInput"))
    src_idx = nc.dram_tensor("src_idx", [128, d["IDXC"]], I16, kind="ExternalInput")
    dlf = nc.dram_tensor("dlf", [128, d["NT"]], F16, kind="ExternalInput")
    iota_c = nc.dram_tensor("iota_c", [128, CHUNK, 128], F16, kind="ExternalInput")
    ident = nc.dram_tensor("ident", [128, 128], F32, kind="ExternalInput")
    ident16 = nc.dram_tensor("ident16", [128, 128], F16, kind="ExternalInput")
    y_out = nc.dram_tensor("y", [NLOC, cfg["OUT"]], F32, kind="ExternalOutput")

    # ---- internal DRAM record tables
    warm_in = nc.dram_tensor("warm_in", [1, 128], F16)
    warm_out = nc.dram_tensor("warm_out", [CORES, 128], F16, addr_space="Shared")
    rec_slice = [nc.dram_tensor(f"rec_slice{i}", [NLOC, L["REC"]], F16)
                 for i, L in enumerate(Ls)]
    rec_table = [nc.dram_tensor(f"rec_table{i}", [N, L["REC"]], F16,
                                addr_space="Shared")
                 for i, L in enumerate(Ls)]

    groups = [list(range(CORES))]

    with tile.TileContext(nc) as tc:
        with (
            tc.tile_pool(name="const", bufs=1) as const,
            tc.tile_pool(name="xt", bufs=1) as xtp,
            tc.tile_pool(name="work", bufs=1) as work,
            tc.tile_pool(name="gp", bufs=3) as gp,
            tc.tile_pool(name="gw", bufs=2) as gwp,
            tc.tile_pool(name="st", bufs=2) as stp,
            tc.tile_pool(name="small", bufs=3) as small,
            tc.tile_pool(name="ps", bufs=2, space="PSUM") as ps,
        ):
            nidx_reg = nc.gpsimd.to_reg(CE)
            nidx_reg2 = nc.gpsimd.to_reg(2 * CE)

            # absorb the one-time collective trigger warmup (~38us) before
            # the first real AllGather
            wz = const.tile([1, 128], F16, name="wz", tag="wz")
            nc.vector.memset(wz[:], 0.0)
            nc.sync.dma_start(warm_in[:], wz[:])
            nc.gpsimd.collective_compute(
                "AllGather", mybir.AluOpType.bypass,
                replica_groups=[list(range(CORES))],
                ins=[warm_in[:]], outs=[warm_out[:]])

            # ---- constants into SBUF
            def load_const(ap, shape, dt=F32, name="cst"):
                t = const.tile(shape, dt, name=name, tag=name)
                nc.sync.dma_start(t[:], ap[:])
                return t

            src_t = load_const(src_idx, [128, d["IDXC"]], I16, name="src_t")
            dlf_t = load_const(dlf, [128, d["NT"]], F16, name="dlf_t")
            iota_t = load_const(iota_c, [128, CHUNK, 128], F16, name="iota_t")
            id_t = load_const(ident, [128, 128], name="id_t")
            id16_t = load_const(ident16, [128, 128], F16, name="id16_t")

            def load_kt(ap, kt, width, name):  # [kt,128,w] dram -> [128,kt,w]
                t = const.tile([128, kt, width], F16, name=name, tag=name)
                nc.sync.dma_start(t[:], ap.rearrange("k p w -> p k w"))
                return t

            W_t = [load_kt(Wp[i], Ls[i]["KT"], Ls[i]["D"], f"W_t{i}")
                   for i in range(3)]
            WA_t = [load_kt(WAp[i], Ls[i]["KT"], 2 * Ls[i]["H"], f"WA_t{i}")
                    for i in range(3)]
            b_t = [load_const(brep[i], [128, Ls[i]["D"]], name=f"b_t{i}")
                   if has_bias[i] else None for i in range(3)]

            # layer-1 x^T
            xT = [xtp.tile([128, NPAD], F16, tag=f"xt{k}", name=f"xTa{k}")
                  for k in range(Ls[0]["KT"])]
            for k in range(Ls[0]["KT"]):
                nc.sync.dma_start(xT[k][:], xT0[k][:])

            y_sb = work.tile([128, MT, cfg["OUT"]], F32, tag="y_sb")
            edl = work.tile([128, MT, 8], F16, tag="edl")
            gidx = [0]  # global gather counter: queue must track the DMASW
                        # sem lane rotation (emission order mod 8 -> mod 4)

            rec_sbs = {}

            def phase_a(li, m, xTl):
                """Dense transform + record/edl write for one m-tile."""
                L = Ls[li]
                H, D, KT, REC, HW16 = L["H"], L["D"], L["KT"], L["REC"], L["HW16"]
                rec_sb = rec_sbs[li]
                ph = ps.tile([128, D], F32, tag="big")
                pe = ps.tile([128, 2 * H], F32, tag="sm")
                lhs = [xTl[k][:, m * 128:(m + 1) * 128] for k in range(KT)]
                for k in range(KT):
                    nc.tensor.matmul(ph[:], lhs[k], W_t[li][:, k, :],
                                     start=(k == 0), stop=(k == KT - 1))
                for k in range(KT):
                    nc.tensor.matmul(pe[:], lhs[k], WA_t[li][:, k, :],
                                     start=(k == 0), stop=(k == KT - 1))
                if L["FP8"]:
                    nc.scalar.activation(
                        rec_sb[:, m, H:H + HW16].bitcast(F8), ph[:],
                        mybir.ActivationFunctionType.Identity)
                else:
                    nc.scalar.activation(
                        rec_sb[:, m, H:H + HW16], ph[:],
                        mybir.ActivationFunctionType.Identity)
                nc.scalar.activation(rec_sb[:, m, 0:H], pe[:, H:2 * H],
                                     mybir.ActivationFunctionType.Identity)
                nc.scalar.activation(edl[:, m, 0:H], pe[:, 0:H],
                                     mybir.ActivationFunctionType.Identity)
                # record write for this m-tile: rows n = m*128 + p
                rs = rec_slice[li]
                if m < MT - 1:
                    nc.sync.dma_start(
                        rs[m * 128:(m + 1) * 128, :]
                        .rearrange("(u p) c -> p u c", p=128),
                        rec_sb[:, m:m + 1, :])
                else:
                    nc.sync.dma_start(rs[m * 128:NLOC, :],
                                      rec_sb[0:LASTM, m, :])

            def alloc_rec(li):
                L = Ls[li]
                rec_sb = work.tile([128, MT, L["REC"]], F16, tag=f"rec_sb{li}",
                                   name=f"rec_sb{li}")
                rec_sbs[li] = rec_sb
                PAD0 = L["H"] + L["HW16"]
                if L["REC"] > PAD0:
                    nc.vector.memset(rec_sb[:, :, PAD0:L["REC"]], 0.0)

            # layer-1 dense phase up front
            alloc_rec(0)
            for m in range(MT):
                phase_a(0, m, xT)

            for li, L in enumerate(Ls):
                H, D, FH, KT, REC = L["H"], L["D"], L["FH"], L["KT"], L["REC"]
                HW16 = L["HW16"]

                # ---------------- Phase B: AllGather -----------------------
                # split: m-tiles 0..MT-2 are written early (their phase_a ran
                # inside the previous layer's loop), so that collective can
                # overlap the previous layer's tail; only the last m-tile's
                # small collective is on the critical path.
                SPL = (MT - 1) * 128
                nc.gpsimd.collective_compute(
                    "AllGather", mybir.AluOpType.bypass,
                    replica_groups=groups,
                    ins=[rec_slice[li][0:SPL, :]],
                    outs=[rec_table[li][0:CORES * SPL, :]],
                )
                nc.gpsimd.collective_compute(
                    "AllGather", mybir.AluOpType.bypass,
                    replica_groups=groups,
                    ins=[rec_slice[li][SPL:NLOC, :]],
                    outs=[rec_table[li][CORES * SPL:N, :]],
                )

                # ---------------- Phase C: edge pipeline -------------------
                if li < 2:
                    KTn = Ls[li + 1]["KT"]
                    xTn = [xtp.tile([128, NPAD], F16, tag=f"xt{k}",
                                    name=f"xTn{li}_{k}") for k in range(KTn)]
                    alloc_rec(li + 1)
                # grouped gathers: 4 chunks share one tile so all 4 become
                # ready together and the Q7 DGE generates them in one launch
                GRP = 4
                # narrow layer-3 records fit 2 chunks per gather slot -> half
                # the Q7 launch overhead there
                CPG = 1
                gtiles = {}
                for c0 in range(0, NCH, CPG * GRP):
                    G4 = gp.tile([128, GRP, CHUNK * 384], F16, tag="G4")
                    for j in range(GRP):
                        cs = c0 + j * CPG
                        if cs >= NCH:
                            break
                        q = gidx[0] % 4
                        gidx[0] += 1
                        gv = (G4[:, j, 0:CPG * CHUNK * REC]
                              .rearrange("p (t r) -> p t r", t=CPG * CHUNK))
                        for k in range(CPG):
                            gtiles[cs + k] = gv[:, k * CHUNK:(k + 1) * CHUNK, :]
                        nc.gpsimd.dma_gather(
                            out_ap=gv,
                            in_ap=rec_table[li][:, :],
                            idxs_ap=src_t[:, cs * IPC:(cs + CPG) * IPC],
                            num_idxs=CPG * CE,
                            num_idxs_reg=nidx_reg if CPG == 1 else nidx_reg2,
                            elem_size=REC,
                            single_packet=False,
                            queue_num=q,
                        )
                # stage1(c): S_T build (DVE), S = S_T^T (PE+Scalar), edv (PE)
                staged = {}

                def stage1(c):
                    # S[e, dstoff] one-hot directly on DVE
                    S = stp.tile([128, CHUNK, 128], F16, tag="S")
                    nc.vector.tensor_tensor(
                        S[:], iota_t[:],
                        dlf_t[:, c * CHUNK:(c + 1) * CHUNK, None]
                        .broadcast_to((128, CHUNK, 128)),
                        mybir.AluOpType.is_equal)
                    # S_T = S^T via PE transposes + Scalar PSUM->SBUF copies
                    ST = stp.tile([128, CE], F16, tag="ST")
                    for j in range(CHUNK // 4):
                        tp4 = ps.tile([128, 4, 128], F16, tag="tp")
                        for t4 in range(4):
                            t = j * 4 + t4
                            nc.tensor.transpose(
                                tp4[:, t4, :], S[:, t, :], id16_t[:])
                        nc.scalar.activation(
                            ST[:, j * 512:(j + 1) * 512]
                            .rearrange("p (a b) -> p a b", a=4),
                            tp4[:],
                            mybir.ActivationFunctionType.Identity)
                    edv = ps.tile([128, CHUNK, H], F32, tag="edv")
                    for t in range(CHUNK):
                        nc.tensor.matmul(
                            edv[:, t, :],
                            ST[:, t * 128:(t + 1) * 128],
                            edl[:, c // CPB, 0:H],
                            start=True, stop=True)
                    staged[c] = (S, edv)

                def emit_epi(blk, acc, den):
                    dene = small.tile([128, H], F32, tag="dene")
                    nc.vector.tensor_scalar_add(dene[:], den[:], DEN_EPS)
                    recip = small.tile([128, H], F32, tag="recip")
                    nc.vector.reciprocal(recip[:], dene[:])
                    if li < 2:
                        h_blk = small.tile([128, D], F32, tag="h_blk",
                                           name=f"h_blk{li}")
                        oview = h_blk[:].rearrange("p (h f) -> p h f", h=H)
                        nc.vector.tensor_tensor(
                            oview, acc[:].rearrange("p (h f) -> p h f", h=H),
                            recip[:, :, None].broadcast_to((128, H, FH)),
                            mybir.AluOpType.mult)
                        if has_bias[li]:
                            nc.vector.tensor_tensor(
                                h_blk[:], h_blk[:], b_t[li][:],
                                mybir.AluOpType.add)
                        nc.scalar.activation(h_blk[:], h_blk[:],
                                             mybir.ActivationFunctionType.Relu)
                        # next layer x^T for this m-tile + its dense phase
                        for k in range(KTn):
                            tp = ps.tile([128, 128], F32, tag="tp")
                            nc.tensor.transpose(
                                tp[:], h_blk[:, k * 128:(k + 1) * 128], id_t[:])
                            nc.scalar.activation(
                                xTn[k][:, blk * 128:(blk + 1) * 128], tp[:],
                                mybir.ActivationFunctionType.Identity)
                        phase_a(li + 1, blk, xTn)
                    else:
                        if has_bias[li]:
                            t3 = small.tile([128, cfg["OUT"]], F32, tag="t3")
                            nc.scalar.activation(
                                t3[:], acc[:],
                                mybir.ActivationFunctionType.Identity,
                                scale=recip[:, 0:1])
                            nc.vector.tensor_tensor(y_sb[:, blk, :], t3[:],
                                                    b_t[li][:],
                                                    mybir.AluOpType.add)
                        else:
                            nc.scalar.activation(
                                y_sb[:, blk, :], acc[:],
                                mybir.ActivationFunctionType.Identity,
                                scale=recip[:, 0:1])

                stage1(0)
                pending = None
                for blk in range(B):
                    acc = ps.tile([128, D], F32, tag="big")
                    den = ps.tile([128, H], F32, tag="sm")
                    for cc in range(CPB):
                        c = blk * CPB + cc
                        G = gtiles[c]
                        S, edv = staged.pop(c)
                        # ex = exp(leaky_relu(es[src] + ed[dst]))
                        z = small.tile([128, CHUNK, H], F32, tag="z")
                        nc.vector.tensor_tensor(
                            z[:], G[:, :, 0:H], edv[:],
                            mybir.AluOpType.add)
                        z2 = small.tile([128, CHUNK, H], F32, tag="z2")
                        nc.vector.scalar_tensor_tensor(
                            z2[:], z[:], NEG_SLOPE, z[:],
                            mybir.AluOpType.mult, mybir.AluOpType.max)
                        ex = small.tile([128, CHUNK, H], F16, tag="ex")
                        nc.scalar.activation(
                            ex[:].rearrange("p a b -> p (a b)"),
                            z2[:].rearrange("p a b -> p (a b)"),
                            mybir.ActivationFunctionType.Exp)
                        # build next chunk's one-hots while Scalar runs exp
                        if c + 1 < NCH:
                            stage1(c + 1)
                        # fused h upcast (fp8) + ex weighting in one DVE op
                        Gh = gwp.tile([128, CHUNK, D], F16, tag="Gh")
                        hsrc = (G[:, :, H:H + HW16].bitcast(F8) if L["FP8"]
                                else G[:, :, H:H + HW16])
                        nc.vector.tensor_tensor(
                            Gh[:].rearrange("p t (h f) -> p t h f", h=H),
                            hsrc.rearrange("p t (h f) -> p t h f", h=H),
                            ex[:, :, :, None].broadcast_to((128, CHUNK, H, FH)),
                            mybir.AluOpType.mult)
                        if cc == 0 and pending is not None:
                            emit_epi(*pending)
                            pending = None
                        for t in range(CHUNK):
                            first = (cc == 0 and t == 0)
                            last = (cc == CPB - 1 and t == CHUNK - 1)
                            nc.tensor.matmul(acc[:], S[:, t, :],
                                             Gh[:, t, :],
                                             start=first, stop=last)
                            nc.tensor.matmul(den[:], S[:, t, :], ex[:, t, :],
                                             start=first, stop=last)

                    # epilogue deferred one block (emitted inside next
                    # block's loop so DVE never stalls on this block's PSUM)
                    pending = (blk, acc, den)
                if pending is not None:
                    emit_epi(*pending)
                    pending = None

                # final log-softmax over all m-tiles in one batch (layer 3)
                if li == 2:
                    mxB = small.tile([128, MT, 1], F32, tag="mxB")
                    nc.vector.tensor_reduce(mxB[:], y_sb[:], mybir.AxisListType.X,
                                            mybir.AluOpType.max)
                    nc.vector.tensor_tensor(
                        y_sb[:], y_sb[:],
                        mxB[:, :, 0:1].broadcast_to((128, MT, cfg["OUT"])),
                        mybir.AluOpType.subtract)
                    escB = small.tile([128, MT, cfg["OUT"]], F32, tag="escB")
                    nc.scalar.activation(
                        escB[:].rearrange("p a b -> p (a b)"),
                        y_sb[:].rearrange("p a b -> p (a b)"),
                        mybir.ActivationFunctionType.Exp)
                    smB = small.tile([128, MT, 1], F32, tag="smB")
                    nc.vector.tensor_reduce(smB[:], escB[:], mybir.AxisListType.X,
                                            mybir.AluOpType.add)
                    lnB = small.tile([128, MT, 1], F32, tag="lnB")
                    nc.scalar.activation(lnB[:, :, 0], smB[:, :, 0],
                                         mybir.ActivationFunctionType.Ln)
                    nc.vector.tensor_tensor(
                        y_sb[:], y_sb[:],
                        lnB[:, :, 0:1].broadcast_to((128, MT, cfg["OUT"])),
                        mybir.AluOpType.subtract)

            # ---- output
            full = MT - 1
            if full:
                nc.sync.dma_start(
                    y_out[0:full * 128, :].rearrange("(m p) c -> p m c", p=128),
                    y_sb[:, 0:full, :])
            nc.sync.dma_start(y_out[full * 128:NLOC, :], y_sb[0:LASTM, full, :])

    nc.compile()
    return nc


# --------------------------------------------------------------------------
# Host-side emulation of the exact device algorithm (for testing)
# --------------------------------------------------------------------------
def emulate(inputs, cfg):
    import ml_dtypes
    d = derived(cfg)
    x = np.asarray(inputs["x"], np.float32)
    ei = np.asarray(inputs["edge_index"])
    N = cfg["N"]
    loop = np.arange(N, dtype=np.int64)
    src = np.concatenate([np.asarray(ei[0], np.int64), loop])
    dst = np.concatenate([np.asarray(ei[1], np.int64), loop])

    W = [np.asarray(inputs[f"W{i}"], np.float32) for i in (1, 2, 3)]
    As = [np.asarray(inputs[f"a{i}s"], np.float32) for i in (1, 2, 3)]
    Ad = [np.asarray(inputs[f"a{i}d"], np.float32) for i in (1, 2, 3)]
    bs = [np.asarray(inputs[f"b{i}"], np.float32) for i in (1, 2, 3)]

    def q16(a): return np.asarray(a, np.float16).astype(np.float32)
    def q8(a): return np.asarray(a, ml_dtypes.float8_e4m3).astype(np.float32)

    h = x
    for li, L in enumerate(d["L"]):
        Wr = W[li].reshape(L["K"], L["H"], L["FH"])
        WAs = np.einsum("ihf,hf->ih", Wr, As[li])
        WAd = np.einsum("ihf,hf->ih", Wr, Ad[li])
        h16 = q16(h)
        hh = h16 @ q16(W[li])
        es = q16(h16 @ q16(WAs))
        ed = q16(h16 @ q16(WAd))
        hq = q8(hh) if L["FP8"] else q16(hh)
        z = es[src] + ed[dst]
        ex = q16(np.exp(np.maximum(z, NEG_SLOPE * z)))
        gq = q16(hq[src].reshape(-1, L["H"], L["FH"]) * ex[:, :, None])
        acc = np.zeros((N, L["H"], L["FH"]), np.float64)
        den = np.zeros((N, L["H"]), np.float64)
        np.add.at(acc, dst, gq)
        np.add.at(den, dst, ex)
        out = (acc / (den[:, :, None] + DEN_EPS)).reshape(N, L["D"]).astype(np.float32)
        out = out + bs[li]
        if li < 2:
            h = np.maximum(out, 0.0)
        else:
            h = out
    m = h.max(axis=1, keepdims=True)
    s = h - m
    return s - np.log(np.exp(s).sum(axis=1, keepdims=True))


# --------------------------------------------------------------------------
# In-map assembly + entry point
# --------------------------------------------------------------------------
def build_in_maps(inputs, cfg):
    shared = prep_weights(inputs, cfg)
    percore = prep_edges(inputs["edge_index"], cfg)
    in_maps = []
    for c in range(cfg["CORES"]):
        m = dict(shared)
        m.update(percore[c])
        m["xT0"] = prep_x(np.asarray(inputs["x"], np.float32), cfg, c)
        in_maps.append(m)
    return in_maps


def pick_t_blk(edge_index, cfg):
    """Smallest CHUNK-multiple tile count that fits the largest dst block."""
    N, CORES, NLOC = cfg["N"], cfg["CORES"], cfg["NLOC"]
    dst = np.concatenate([np.asarray(edge_index[1], np.int64),
                          np.arange(N, dtype=np.int64)])
    blk = (dst % NLOC) // 128 + (dst // NLOC) * ((NLOC + 127) // 128)
    mx = int(np.bincount(blk).max())
    ch = cfg["CHUNK"]
    tiles = (mx + 127) // 128
    return max(((tiles + ch - 1) // ch) * ch, ch)


_PROGRAM_CACHE = {}
LAST_EXEC_NS = None


def kernel(**inputs):
    global LAST_EXEC_NS
    cfg = full_cfg()
    t_blk = pick_t_blk(inputs["edge_index"], cfg)
    cfg = full_cfg(t_blk)
    has_bias = tuple(bool(np.any(np.asarray(inputs[f"b{i}"]))) for i in (1, 2, 3))
    key = ("full", t_blk, has_bias)
    if key not in _PROGRAM_CACHE:
        _PROGRAM_CACHE[key] = build_program(cfg, has_bias)
    nc = _PROGRAM_CACHE[key]
    in_maps = build_in_maps(inputs, cfg)
    res = run_bass_kernel_spmd(nc, in_maps, core_ids=list(range(cfg["CORES"])))
    LAST_EXEC_NS = res.exec_time_ns
    y = np.concatenate([res.results[c]["y"] for c in range(cfg["CORES"])], axis=0)
    return y.astype(np.float32)


def time_kernel(inputs, iters=0):
    """On-device exec time from the NTFF hardware profile (core 0)."""
    import shutil, tempfile
    cfg = full_cfg(pick_t_blk(inputs["edge_index"], full_cfg()))
    has_bias = tuple(bool(np.any(np.asarray(inputs[f"b{i}"]))) for i in (1, 2, 3))
    key = ("full", cfg["T_BLK"], has_bias)
    if key not in _PROGRAM_CACHE:
        _PROGRAM_CACHE[key] = build_program(cfg, has_bias)
    nc = _PROGRAM_CACHE[key]
    in_maps = build_in_maps(inputs, cfg)
    tmpdir = tempfile.mkdtemp(prefix="gat_trace_")
    res = run_bass_kernel_spmd(nc, in_maps, core_ids=list(range(cfg["CORES"])),
                               trace=True, tmpdir=tmpdir, trace_cores=[0])
    return dict(est_exec_s=(res.exec_time_ns or 0) * 1e-9,
                trace_dir=tmpdir,
                profile_json=res.profile_json)


if __name__ == "__main__":
    # quick smoke: build the full program
    nc = build_program(full_cfg())
    print("program built ok")
